# revision 1
# baseline (speedup 1.0000x reference)
"""CKAN (gnn_message_passing) Trainium2 kernel, v2.

Data-parallel over 8 NeuronCores (512 batch rows each), no collectives.
Two-level entity fetch (dma_gather idx are int16, N_ENTITY=100000):
  phase A: sorted-unique h∪t rows gathered from the full table with
           windowed calls, converted to f16, staged to HBM as [na, 128]
           rows (64 real features + 64 pad);
  phase B: h rows gathered TRANSPOSE-mode (feature-major [64, cols] x
           straight into the MLP, no PE transposes); t rows gathered
           batch-major for the weighted sum.
Relation one-hots are host-built and streamed ([32, n_col] f16), so the
first-layer matmul is w1h.T@x + r1p.T@onehot.  The gate-L2 / att-L3
"flip" matmul (lhsT = relu activations, rhs = [gate_w2 | att_w3]) gives
batch-major [128b, 65] = [gate2 | s3] per (t, q).  Softmax over
neighbors skips the max-subtraction (sigmoid outputs are bounded), so
halves combine with plain unnormalized sums:
  att = exp(s3) / sum_t exp(s3),  out = (sum_t exp*gate2*t_e) * 2/sum.
Columns are t-major (col = t*512 + b).
"""
import sys
sys.path.insert(0, '/opt/trn_rl_repo')
import numpy as np

# ---- problem dims ----
DIM = 64
N_ENTITY = 100000
N_RELATION = 32
N_LAYER = 2
B = 4096
T = 32
N_CORES = 8
WIN = 32768
_NC_CACHE = None


def _dims():
    b_core = B // N_CORES
    n_col = b_core * T
    nwin = (N_ENTITY + WIN - 1) // WIN
    import math
    caps = []
    for w in range(nwin):
        width = min(WIN, N_ENTITY - WIN * w)
        mean = width * (1.0 - (1.0 - 1.0 / N_ENTITY) ** (2 * n_col))
        cap = int(mean + 8 * math.sqrt(max(mean, 1.0)) + 64)
        caps.append(-(-cap // 128) * 128)
    na = sum(caps)
    assert na % 128 == 0 and na <= 32768
    return b_core, n_col, nwin, caps, na


def _wrap_idx16(a):
    """int16 vector -> dma_gather idx layout [128, ceil(n/16)]."""
    a = np.asarray(a, dtype=np.int16)
    n = len(a)
    pad = (-n) % 16
    if pad:
        a = np.concatenate([a, np.full(pad, -1, np.int16)])
    w = a.reshape(-1, 16).T.copy()
    return np.tile(w, (8, 1))


def _host_prep_tl(h_flat, t_flat):
    b_core, n_col, nwin, caps, na = _dims()
    uni = np.unique(np.concatenate([h_flat, t_flat]))
    val_to_pos = np.full(N_ENTITY, -1, np.int32)
    idxA_parts = []
    off = 0
    for w in range(nwin):
        lo, hi = WIN * w, min(WIN * (w + 1), N_ENTITY)
        seg = uni[(uni >= lo) & (uni < hi)]
        cap = caps[w]
        assert len(seg) <= cap, f"window {w} overflow: {len(seg)} > {cap}"
        val_to_pos[seg] = off + np.arange(len(seg), dtype=np.int32)
        assert len(seg) == 0 or off + len(seg) - 1 <= 32767, "position overflow"
        fill = (seg[-1] - lo) if len(seg) else 0
        seg_l = np.concatenate([(seg - lo).astype(np.int16),
                                np.full(cap - len(seg), fill, np.int16)])
        idxA_parts.append(seg_l)
        off += cap
    idxA = np.concatenate(idxA_parts)
    h_loc = val_to_pos[h_flat]
    t_loc = val_to_pos[t_flat]
    assert (h_loc >= 0).all() and (t_loc >= 0).all()
    return idxA, h_loc.astype(np.int16), t_loc.astype(np.int16)


TL_LIST = [("u", 0), ("u", 1), ("i", 0), ("i", 1)]


def _build_program():
    import concourse.bacc as bacc
    import concourse.tile as tile
    from concourse import mybir
    from concourse.masks import make_identity
    from concourse import tile_sem_assignment as tsa

    # Tile assigns DMASW sem lanes round-robin in scheduled order, but each
    # lane is hardware-locked to SWDGE queue (lane % 4).  Force gather
    # instructions onto lanes consistent with their queue_num.
    if not getattr(tsa.TileClockTick, "_gather_lane_patched", False):
        _orig_assign_tick = tsa.TileClockTick._assign_tick

        def _patched_assign_tick(self, inst):
            if isinstance(inst, mybir.InstDMAGatherAnt):
                q = inst.queue_num
                tog = getattr(self, "_gather_lane_toggle", None)
                if tog is None:
                    tog = self._gather_lane_toggle = {}
                k = tog.get(q, 0)
                tog[q] = k ^ 1
                saved = self.next_sw_dma_idx
                self.next_sw_dma_idx = q + 4 * k
                try:
                    return _orig_assign_tick(self, inst)
                finally:
                    self.next_sw_dma_idx = saved
            return _orig_assign_tick(self, inst)

        tsa.TileClockTick._assign_tick = _patched_assign_tick
        tsa.TileClockTick._gather_lane_patched = True

    f32 = mybir.dt.float32
    f16 = mybir.dt.float16
    i16 = mybir.dt.int16
    AF = mybir.ActivationFunctionType
    ALU = mybir.AluOpType
    AX = mybir.AxisListType

    b_core, n_col, nwin, caps, na = _dims()
    NB = b_core // 128               # b-chunks (4)
    NH = 4                           # quarters
    TH = T // NH                     # t per quarter (8)
    GQ = b_core * TH                 # columns per quarter = per B-gather call (4096)
    QCOL = n_col // 8                # onehot chunk (2048 cols)

    nc = bacc.Bacc("TRN2", target_bir_lowering=False, debug=True,
                   num_swdge_queues=4)
    _qctr = [0]

    def _nextq():
        q = _qctr[0] % 4
        _qctr[0] += 1
        return q

    ent = nc.dram_tensor("ent", [N_ENTITY, DIM], f32, kind="ExternalInput")
    ent16 = nc.dram_tensor("ent16", [N_ENTITY, 2 * DIM], f16, kind="ExternalInput")
    w1h = nc.dram_tensor("w1h", [64, 128], f16, kind="ExternalInput")
    r1p = nc.dram_tensor("r1p", [N_RELATION, 128], f16, kind="ExternalInput")
    w2 = nc.dram_tensor("w2", [64, 64], f16, kind="ExternalInput")
    wflip = nc.dram_tensor("wflip", [128, 65], f16, kind="ExternalInput")
    items16 = nc.dram_tensor("items16", [128, max(b_core // 16, 1)], i16,
                             kind="ExternalInput")
    idxA, idxh, idxt, r1h = {}, {}, {}, {}
    for k in range(4):
        idxA[k] = nc.dram_tensor(f"idxA{k}", [128, na // 16], i16, kind="ExternalInput")
        idxh[k] = nc.dram_tensor(f"idxh{k}", [128, n_col // 16], i16, kind="ExternalInput")
        idxt[k] = nc.dram_tensor(f"idxt{k}", [128, n_col // 16], i16, kind="ExternalInput")
        r1h[k] = nc.dram_tensor(f"r1h{k}", [N_RELATION, n_col], f16, kind="ExternalInput")
    scores_hbm = nc.dram_tensor("scores", [b_core], f32, kind="ExternalOutput")

    # ---- phase-A call list (shared by all TLs) ----
    acalls = []
    off = 0
    for w in range(nwin):
        nw_ = caps[w]
        nsplit = max(1, -(-nw_ // 3968))
        step = -(-(-(-nw_ // nsplit)) // 128) * 128
        done = 0
        while done < nw_:
            nn = min(step, nw_ - done)
            acalls.append((w, off + done, nn))
            done += nn
        off += nw_
    NPIECE = 8
    bounds = [0]
    for pi in range(1, NPIECE):
        tgt = na * pi // NPIECE
        bb = 0
        for (_, o2, nn) in acalls:
            if o2 >= tgt:
                break
            bb = o2 + nn
        bounds.append(bb)
    bounds.append(na)
    piece_slots = max(-(-(bounds[pi + 1] - bounds[pi]) // 128)
                      for pi in range(NPIECE))

    with tile.TileContext(nc) as tc:
        with (
            tc.tile_pool(name="dram", bufs=1, space="DRAM") as dp,
            tc.tile_pool(name="const", bufs=1) as cp,
            tc.tile_pool(name="apool", bufs=2) as ap,
            tc.tile_pool(name="bg", bufs=4) as bg,
            tc.tile_pool(name="gsp", bufs=2) as gsp,
            tc.tile_pool(name="rhp", bufs=2) as rhp,
            tc.tile_pool(name="wp", bufs=2) as wp,
            tc.tile_pool(name="sp1", bufs=1) as sp1,
            tc.tile_pool(name="iap", bufs=2) as iap,
            tc.tile_pool(name="ihp", bufs=2) as ihp,
            tc.tile_pool(name="eup", bufs=2) as eup,
            tc.tile_pool(name="kp", bufs=1) as kp,
            tc.tile_pool(name="psP1", bufs=2, space="PSUM") as psP1,
            tc.tile_pool(name="psP2", bufs=2, space="PSUM") as psP2,
            tc.tile_pool(name="psPF", bufs=2, space="PSUM") as psPF,
        ):
            # partition-major staging: token j = (i%128)*nslots + i//128 lives
            # at stage[j // nslots, j % nslots, :]; rows stay 256B-linear in j
            # while phase-A writes become 128 big contiguous descriptors.
            nslots = na // 128
            stage = [dp.tile([128, nslots, 2 * DIM], f16, name=f"stage{k}")
                     for k in range(4)]
            stage_rows = [s[:].rearrange("p s d -> (p s) d") for s in stage]

            ident = cp.tile([128, 128], f32)
            make_identity(nc, ident[:])
            w1h_sb = cp.tile([64, 128], f16)
            nc.sync.dma_start(out=w1h_sb[:], in_=w1h[:])
            r1p_sb = cp.tile([N_RELATION, 128], f16)
            nc.sync.dma_start(out=r1p_sb[:], in_=r1p[:])
            w2_sb = cp.tile([64, 64], f16)
            nc.sync.dma_start(out=w2_sb[:], in_=w2[:])
            wflip_sb = cp.tile([128, 65], f16)
            nc.sync.dma_start(out=wflip_sb[:], in_=wflip[:])

            items_sb = cp.tile([128, max(b_core // 16, 1)], i16)
            nc.sync.dma_start(out=items_sb[:], in_=items16[:])
            iorig = kp.tile([128, NB, DIM], f32)
            nc.gpsimd.dma_gather(
                out_ap=iorig[:], in_ap=ent[:], idxs_ap=items_sb[:],
                num_idxs=b_core, num_idxs_reg=b_core, elem_size=DIM,
                queue_num=_nextq(), single_packet=False)

            def phase_a(k):
                ia = iap.tile([128, na // 16], i16, name=f"ia{k}", tag="ia")
                nc.sync.dma_start(out=ia[:], in_=idxA[k][:])
                for piece in range(NPIECE):
                    p_lo, p_hi = bounds[piece], bounds[piece + 1]
                    if p_lo >= p_hi:
                        continue
                    a16 = ap.tile([128, piece_slots, 2 * DIM], f16,
                                  name=f"a16_{k}_{piece}", tag="a16")
                    for (w, o2, nn) in acalls:
                        if o2 >= p_hi or o2 + nn <= p_lo:
                            continue
                        assert o2 >= p_lo and o2 + nn <= p_hi, "call straddles piece"
                        ol = o2 - p_lo
                        nc.gpsimd.dma_gather(
                            out_ap=a16[:, ol // 128:(ol + nn) // 128, :],
                            in_ap=ent16[WIN * w: min(WIN * (w + 1), N_ENTITY), :],
                            idxs_ap=ia[:, o2 // 16:(o2 + nn) // 16],
                            num_idxs=nn, num_idxs_reg=nn, elem_size=2 * DIM,
                            queue_num=_nextq(), single_packet=False)
                    ns = (p_hi - p_lo) // 128
                    # engine-mediated relay: SWDGE<->HWDGE ordering is only
                    # reliable through engine ops, so copy the real half into
                    # a16b (scalar) and let the staging DMA read that.
                    a16b = ap.tile([128, piece_slots, 2 * DIM], f16,
                                   name=f"a16b_{k}_{piece}", tag="a16b")
                    nc.scalar.activation(out=a16b[:, 0:ns, 0:DIM],
                                         in_=a16[:, 0:ns, 0:DIM], func=AF.Copy)
                    nc.sync.dma_start(
                        out=stage[k][:, p_lo // 128:p_hi // 128, :],
                        in_=a16b[:, 0:ns, :])

            otl = [None] * 4            # per-TL [128, NB, DIM] outputs
            uob = None                  # user-origin batch-major

            phase_a(0)
            phase_a(1)

            for k, (tw, l) in enumerate(TL_LIST):
                ih = ihp.tile([128, n_col // 16], i16, name=f"ih{k}", tag="ih")
                nc.sync.dma_start(out=ih[:], in_=idxh[k][:])
                it = ihp.tile([128, n_col // 16], i16, name=f"it{k}", tag="it")
                nc.sync.dma_start(out=it[:], in_=idxt[k][:])

                E = eup.tile([128, T, NB], f32, name=f"E{k}", tag="E")
                U = eup.tile([128, NB, DIM], f32, name=f"U{k}", tag="U")
                if tw == "u" and l == 0:
                    xsum = kp.tile([64, b_core], f32)

                hxs, tes = [], []
                for hf in range(NH):
                    cbase = hf * GQ
                    hx = bg.tile([128, 1, GQ], f16, name=f"hx{k}_{hf}", tag="hx")
                    te = bg.tile([128, GQ // 128, 2 * DIM], f16,
                                 name=f"te{k}_{hf}", tag="te")
                    nc.gpsimd.dma_gather(
                        out_ap=hx[:],
                        in_ap=stage_rows[k],
                        idxs_ap=ih[:, cbase // 16:(cbase + GQ) // 16],
                        num_idxs=GQ, num_idxs_reg=GQ, elem_size=2 * DIM,
                        transpose=True,
                        queue_num=_nextq(), single_packet=False)
                    nc.gpsimd.dma_gather(
                        out_ap=te[:],
                        in_ap=stage_rows[k],
                        idxs_ap=it[:, cbase // 16:(cbase + GQ) // 16],
                        num_idxs=GQ, num_idxs_reg=GQ, elem_size=2 * DIM,
                        queue_num=_nextq(), single_packet=False)
                    hxs.append(hx)
                    tes.append(te)

                for hf in range(NH):
                    cbase = hf * GQ
                    hx, te = hxs[hf], tes[hf]
                    hxf = hx[:].rearrange("p a b -> p (a b)")
                    gs = gsp.tile([128, TH, NB, 65], f16, name=f"gs{k}_{hf}", tag="gs")

                    for tt in range(TH):
                        ccol = tt * b_core              # col offset within half
                        if ccol % QCOL == 0:
                            roh = rhp.tile([N_RELATION, QCOL], f16,
                                           name=f"roh{k}_{hf}_{ccol // QCOL}", tag="roh")
                            nc.sync.dma_start(
                                out=roh[:],
                                in_=r1h[k][:, cbase + ccol: cbase + ccol + QCOL])
                        p1 = psP1.tile([128, b_core], f32, space="PSUM",
                                       name="p1", tag="p1")
                        nc.tensor.matmul(out=p1[:], lhsT=w1h_sb[:],
                                         rhs=hxf[0:64, ccol:ccol + b_core],
                                         start=True, stop=False)
                        nc.tensor.matmul(out=p1[:], lhsT=r1p_sb[:],
                                         rhs=roh[:, ccol % QCOL:ccol % QCOL + b_core],
                                         start=False, stop=True)
                        relu1 = wp.tile([128, b_core], f16, name="relu1", tag="relu1")
                        nc.vector.tensor_scalar(
                            out=relu1[0:64, :], in0=p1[0:64, :], scalar1=0.0,
                            scalar2=None, op0=ALU.max)
                        s1a = wp.tile([64, b_core], f16, name="s1a", tag="s1a")
                        nc.scalar.activation(out=s1a[:], in_=p1[64:128, :],
                                             func=AF.Relu)
                        p2 = psP2.tile([64, b_core], f32, space="PSUM",
                                       name="p2", tag="p2")
                        nc.tensor.matmul(out=p2[:], lhsT=w2_sb[:],
                                         rhs=s1a[:], start=True, stop=True)
                        nc.scalar.activation(out=relu1[64:128, :], in_=p2[:],
                                             func=AF.Relu)
                        pf = psPF.tile([128, NB, 65], f32, space="PSUM",
                                       name="pf", tag="pf")
                        for q in range(NB):
                            nc.tensor.matmul(out=pf[:, q, :],
                                             lhsT=relu1[:, q * 128:(q + 1) * 128],
                                             rhs=wflip_sb[:], start=True, stop=True)
                        nc.scalar.activation(out=gs[:, tt, :, :], in_=pf[:],
                                             func=AF.Sigmoid)

                    # ---- per-half epilogue ----
                    nc.scalar.activation(out=E[:, hf * TH:(hf + 1) * TH, :],
                                         in_=gs[:, :, :, 64], func=AF.Exp)
                    prod = wp.tile([128, TH, NB, DIM], f16, name="prod", tag="prod")
                    nc.vector.tensor_tensor(
                        out=prod[:],
                        in0=te[:].rearrange("p (t q) d -> p t q d", q=NB)[:, :, :, 0:DIM],
                        in1=gs[:, :, :, 0:DIM], op=ALU.mult)
                    nc.vector.tensor_tensor(
                        out=prod[:], in0=prod[:],
                        in1=E[:, hf * TH:(hf + 1) * TH, :, None]
                            .to_broadcast([128, TH, NB, DIM]),
                        op=ALU.mult)
                    t8 = wp.tile([128, TH // 2, NB, DIM], f16, name="t8", tag="t8")
                    nc.vector.tensor_add(out=t8[:], in0=prod[:, 0:TH // 2, :, :],
                                         in1=prod[:, TH // 2:TH, :, :])
                    nc.vector.tensor_add(out=t8[:, 0:2, :, :], in0=t8[:, 0:2, :, :],
                                         in1=t8[:, 2:4, :, :])
                    if hf == 0:
                        nc.vector.tensor_add(out=U[:], in0=t8[:, 0, :, :],
                                             in1=t8[:, 1, :, :])
                    else:
                        nc.vector.tensor_add(out=t8[:, 0, :, :],
                                             in0=t8[:, 0, :, :],
                                             in1=t8[:, 1, :, :])
                        nc.vector.tensor_add(out=U[:], in0=U[:],
                                             in1=t8[:, 0, :, :])

                    if tw == "u" and l == 0:
                        # user-origin: sum x over t (feature-major halves)
                        xv = hxf[0:64, :].rearrange("p (t b) -> p t b", t=TH)
                        x8 = sp1.tile([64, TH // 2, b_core], f32, name="x8", tag="x8")
                        nc.vector.tensor_add(out=x8[:], in0=xv[:, 0:TH // 2, :],
                                             in1=xv[:, TH // 2:TH, :])
                        nc.vector.tensor_add(out=x8[:, 0:2, :], in0=x8[:, 0:2, :],
                                             in1=x8[:, 2:4, :])
                        if hf == 0:
                            nc.vector.tensor_add(out=xsum[:], in0=x8[:, 0, :],
                                                 in1=x8[:, 1, :])
                        else:
                            nc.vector.tensor_add(out=x8[:, 0, :],
                                                 in0=x8[:, 0, :],
                                                 in1=x8[:, 1, :])
                            nc.vector.tensor_add(out=xsum[:], in0=xsum[:],
                                                 in1=x8[:, 0, :])

                # ---- TL epilogue: softmax denom + normalized output ----
                e8 = wp.tile([128, 8, NB], f32, name="e8", tag="e8")
                nc.vector.tensor_add(out=e8[:], in0=E[:, 0:8, :], in1=E[:, 8:16, :])
                nc.vector.tensor_add(out=e8[:], in0=e8[:], in1=E[:, 16:24, :])
                nc.vector.tensor_add(out=e8[:], in0=e8[:], in1=E[:, 24:32, :])
                e2 = wp.tile([128, 2, NB], f32, name="e2", tag="e2")
                nc.vector.tensor_add(out=e2[:], in0=e8[:, 0:2, :], in1=e8[:, 2:4, :])
                nc.vector.tensor_add(out=e2[:], in0=e2[:], in1=e8[:, 4:6, :])
                nc.vector.tensor_add(out=e2[:], in0=e2[:], in1=e8[:, 6:8, :])
                e1 = wp.tile([128, 1, NB], f32, name="e1", tag="e1")
                nc.vector.tensor_add(out=e1[:], in0=e2[:, 0:1, :], in1=e2[:, 1:2, :])
                rs = wp.tile([128, 1, NB], f32, name="rs", tag="rs")
                nc.vector.reciprocal(out=rs[:], in_=e1[:])
                o = kp.tile([128, NB, DIM], f32, name=f"otl{k}", tag=f"otl{k}")
                nc.vector.tensor_tensor(
                    out=o[:], in0=U[:],
                    in1=rs[:, 0, :, None].to_broadcast([128, NB, DIM]),
                    op=ALU.mult)
                otl[k] = o

                if tw == "u" and l == 0:
                    uob = kp.tile([128, NB, DIM], f32)
                    for q in range(NB):
                        ups = psPF.tile([128, DIM], f32, space="PSUM",
                                        name="ups", tag="ups")
                        nc.tensor.transpose(out=ups[:],
                                            in_=xsum[:, q * 128:(q + 1) * 128],
                                            identity=ident[0:64, 0:64])
                        nc.scalar.activation(out=uob[:, q, :], in_=ups[:],
                                             func=AF.Copy)

                if k + 2 < 4:
                    phase_a(k + 2)

            # ---- scores ----
            # otl values are unnormalized-by-2: out = U * (1/S); the gate's
            # factor 2 enters squared (u and i towers) => multiply by 4.
            macc = sp1.tile([128, NB, DIM], f32, name="macc", tag="macc")
            nc.vector.tensor_tensor(out=macc[:], in0=uob[:], in1=iorig[:],
                                    op=ALU.mult)
            nc.vector.tensor_scalar(out=macc[:], in0=macc[:], scalar1=1.0 / T,
                                    scalar2=None, op0=ALU.mult)
            for ku, ki in ((0, 2), (1, 3)):
                mu = sp1.tile([128, NB, DIM], f32, name="mu", tag="mu")
                nc.vector.tensor_tensor(out=mu[:], in0=otl[ku][:], in1=otl[ki][:],
                                        op=ALU.mult)
                nc.vector.tensor_scalar(out=mu[:], in0=mu[:], scalar1=4.0,
                                        scalar2=None, op0=ALU.mult)
                nc.vector.tensor_add(out=macc[:], in0=macc[:], in1=mu[:])
            ssum = wp.tile([128, NB, 1], f32, name="ssum", tag="ssum")
            nc.vector.tensor_reduce(out=ssum[:], in_=macc[:], axis=AX.X,
                                    op=ALU.add)
            sc_all = kp.tile([128, NB], f32)
            nc.scalar.activation(out=sc_all[:], in_=ssum[:, :, 0], func=AF.Sigmoid)
            nc.sync.dma_start(out=scores_hbm.rearrange("(s p) -> p s", p=128),
                              in_=sc_all[:])
    nc.compile()
    return nc


def _make_in_maps(inputs):
    b_core, n_col, nwin, caps, na = _dims()
    ent = np.asarray(inputs["ent_emb"], np.float32)
    rel = np.asarray(inputs["rel_emb"], np.float32)
    att_w1 = np.asarray(inputs["att_w1"], np.float32)
    att_w2 = np.asarray(inputs["att_w2"], np.float32)
    att_w3 = np.asarray(inputs["att_w3"], np.float32)
    gate_w1 = np.asarray(inputs["gate_w1"], np.float32)
    gate_w2 = np.asarray(inputs["gate_w2"], np.float32)
    items = np.asarray(inputs["items"]).astype(np.int64)
    idx6 = {n: np.asarray(inputs[n]).astype(np.int64)
            for n in ("user_h", "user_r", "user_t", "item_h", "item_r", "item_t")}

    ent16_pad = np.zeros((N_ENTITY, 2 * DIM), np.float16)
    ent16_pad[:, 0:DIM] = ent.astype(np.float16)
    w1h = np.concatenate([gate_w1[:DIM], att_w1[:DIM]], axis=1).astype(np.float16)
    r1p = (rel @ np.concatenate([gate_w1[DIM:], att_w1[DIM:]], axis=1)).astype(np.float16)
    wflip = np.zeros((128, 65), np.float16)
    wflip[0:64, 0:64] = gate_w2.astype(np.float16)
    wflip[64:128, 64:65] = att_w3.astype(np.float16)

    in_maps = []
    for c in range(N_CORES):
        sl = slice(c * b_core, (c + 1) * b_core)
        im = {
            "ent": ent, "ent16": ent16_pad,
            "w1h": w1h, "r1p": r1p, "w2": att_w2.astype(np.float16),
            "wflip": wflip,
            "items16": _wrap_idx16(items[sl].astype(np.int16)),
        }
        for k, (tw, l) in enumerate(TL_LIST):
            pre = "user" if tw == "u" else "item"
            h = idx6[f"{pre}_h"][l, sl].T.ravel()
            t = idx6[f"{pre}_t"][l, sl].T.ravel()
            r = idx6[f"{pre}_r"][l, sl].T.ravel()
            ia, hl, tl_ = _host_prep_tl(h, t)
            # partition-major staging-table token remap (see _build_program)
            nslots = na // 128
            hl = ((hl.astype(np.int32) % 128) * nslots
                  + hl.astype(np.int32) // 128).astype(np.int16)
            tl_ = ((tl_.astype(np.int32) % 128) * nslots
                   + tl_.astype(np.int32) // 128).astype(np.int16)
            im[f"idxA{k}"] = _wrap_idx16(ia)
            im[f"idxh{k}"] = _wrap_idx16(hl)
            im[f"idxt{k}"] = _wrap_idx16(tl_)
            im[f"r1h{k}"] = (np.arange(N_RELATION)[:, None] == r[None, :]
                             ).astype(np.float16)
        in_maps.append(im)
    return in_maps


def kernel(**inputs):
    global _NC_CACHE
    import os
    from concourse.bass_utils import run_bass_kernel_spmd

    if _NC_CACHE is None:
        _NC_CACHE = _build_program()
    nc = _NC_CACHE
    in_maps = _make_in_maps(inputs)
    trace = bool(int(os.environ.get("CKAN_TRACE", "0")))
    res = run_bass_kernel_spmd(nc, in_maps, core_ids=list(range(N_CORES)),
                               trace=trace)
    if trace and res.exec_time_ns is not None:
        print(f"HW exec time: {res.exec_time_ns} ns")
    if trace and res.instructions_and_trace is not None:
        print(f"trace path: {res.instructions_and_trace[1]}")
    b_core = B // N_CORES
    out = np.concatenate([res.results[c]["scores"] for c in range(N_CORES)])
    return out.astype(np.float32)



# revision 4
# speedup vs baseline: 1.0275x; 1.0275x over previous
"""CKAN (gnn_message_passing) Trainium2 kernel, v3.

Data-parallel over 8 NeuronCores (512 batch rows each), no collectives.
Two-level entity fetch (dma_gather idx are int16, N_ENTITY=100000).

v3 changes over v2:
  - Staged tables are SHARED between the two layers of a tower (2 TLs per
    stage).  The union of h∪t uniques (~48K rows) exceeds the unsigned
    int16 range, so phase-B gathers use an offset base (stage row 32768)
    with SIGNED int16 tokens: the Q7 ucode multiplies idx as the signed
    operand (IVP_MULUSAN_2X32), so negative tokens address below base.
    This cuts phase-A gather descriptors ~16% (dedup across layers).
  - Trailing-negative idxs would be dropped by the ucode's tail scan, so
    every phase-B call is padded to 4224 idxs with a positive dummy token.
  - relu(p1) is ONE vector op over all 128 partitions (gate relu + att
    relu1 were previously split across vector and scalar).
  - Phase A of stage 1 is emitted right after stage-0's phase-B gathers
    so the Pool engine (the bottleneck: SWDGE descriptor generation at
    ~3.2-3.7 ns/idx) never idles.
Columns are t-major (col = t*512 + b).  Softmax skips max-subtraction
(sigmoid-bounded scores): att = exp(s3)/sum, out = (sum exp*gate2*t_e)*2/sum.
"""
import sys
sys.path.insert(0, '/opt/trn_rl_repo')
import numpy as np

# ---- problem dims ----
DIM = 64
N_ENTITY = 100000
N_RELATION = 32
N_LAYER = 2
B = 4096
T = 32
N_CORES = 8
WIN = 32768
_NC_CACHE = None

TL_LIST = [("u", 0), ("u", 1), ("i", 0), ("i", 1)]   # stage = k // 2
BASE_TOK = 32768       # phase-B gather base row (signed idx offset)
PAD_IDX = 128          # per-call idx padding (positive-tail guarantee)


def _dims():
    import math
    b_core = B // N_CORES
    n_col = b_core * T
    nwin = (N_ENTITY + WIN - 1) // WIN
    draws = 4 * n_col          # h+t of BOTH layers of a tower
    caps = []
    for w in range(nwin):
        width = min(WIN, N_ENTITY - WIN * w)
        mean = width * (1.0 - (1.0 - 1.0 / N_ENTITY) ** draws)
        cap = int(mean + 8 * math.sqrt(max(mean, 1.0)) + 64)
        caps.append(-(-cap // 128) * 128)
    na = sum(caps)
    assert na % 128 == 0 and BASE_TOK < na <= 2 * BASE_TOK
    return b_core, n_col, nwin, caps, na


def _wrap_idx16(a):
    """int16 vector -> dma_gather idx layout [128, ceil(n/16)]."""
    a = np.asarray(a, dtype=np.int16)
    n = len(a)
    pad = (-n) % 16
    if pad:
        a = np.concatenate([a, np.full(pad, -1, np.int16)])
    w = a.reshape(-1, 16).T.copy()
    return np.tile(w, (8, 1))


def _host_prep_stage(streams):
    """streams: [h0, t0, h1, t1] flat int64 arrays (t-major column order).
    Returns (idxA, [loc arrays as signed-token int16 per stream])."""
    b_core, n_col, nwin, caps, na = _dims()
    uni = np.unique(np.concatenate(streams))
    val_to_pos = np.full(N_ENTITY, -1, np.int32)
    idxA_parts = []
    off = 0
    for w in range(nwin):
        lo, hi = WIN * w, min(WIN * (w + 1), N_ENTITY)
        seg = uni[(uni >= lo) & (uni < hi)]
        cap = caps[w]
        assert len(seg) <= cap, f"window {w} overflow: {len(seg)} > {cap}"
        val_to_pos[seg] = off + np.arange(len(seg), dtype=np.int32)
        fill = (seg[-1] - lo) if len(seg) else 0
        seg_l = np.concatenate([(seg - lo).astype(np.int16),
                                np.full(cap - len(seg), fill, np.int16)])
        idxA_parts.append(seg_l)
        off += cap
    idxA = np.concatenate(idxA_parts)
    nslots = na // 128
    locs = []
    for s in streams:
        pos = val_to_pos[s]
        assert (pos >= 0).all()
        tok = (pos % 128) * nslots + pos // 128        # partition-major token
        locs.append(np.int16(tok - BASE_TOK))
    return idxA, locs


def _pad_calls(loc16, gq):
    """Split a [n_col] signed-token stream into per-quarter calls padded to
    gq+PAD_IDX with idx 0 (token BASE_TOK, always >= 0: defeats the ucode's
    trailing-negative drop)."""
    parts = []
    for hf in range(len(loc16) // gq):
        parts.append(loc16[hf * gq:(hf + 1) * gq])
        parts.append(np.zeros(PAD_IDX, np.int16))
    return np.concatenate(parts)


def _build_program():
    import concourse.bacc as bacc
    import concourse.tile as tile
    from concourse import mybir
    from concourse.masks import make_identity
    from concourse import tile_sem_assignment as tsa

    # Tile assigns DMASW sem lanes round-robin in scheduled order, but each
    # lane is hardware-locked to SWDGE queue (lane % 4).  Force gather
    # instructions onto lanes consistent with their queue_num.
    if not getattr(tsa.TileClockTick, "_gather_lane_patched", False):
        _orig_assign_tick = tsa.TileClockTick._assign_tick

        def _patched_assign_tick(self, inst):
            if isinstance(inst, mybir.InstDMAGatherAnt):
                q = inst.queue_num
                tog = getattr(self, "_gather_lane_toggle", None)
                if tog is None:
                    tog = self._gather_lane_toggle = {}
                k = tog.get(q, 0)
                tog[q] = k ^ 1
                saved = self.next_sw_dma_idx
                self.next_sw_dma_idx = q + 4 * k
                try:
                    return _orig_assign_tick(self, inst)
                finally:
                    self.next_sw_dma_idx = saved
            return _orig_assign_tick(self, inst)

        tsa.TileClockTick._assign_tick = _patched_assign_tick
        tsa.TileClockTick._gather_lane_patched = True

    f32 = mybir.dt.float32
    f16 = mybir.dt.float16
    i16 = mybir.dt.int16
    AF = mybir.ActivationFunctionType
    ALU = mybir.AluOpType
    AX = mybir.AxisListType

    b_core, n_col, nwin, caps, na = _dims()
    NB = b_core // 128               # b-chunks (4)
    NH = 4                           # quarters
    TH = T // NH                     # t per quarter (8)
    GQ = b_core * TH                 # real columns per B-gather call (4096)
    GQP = GQ + PAD_IDX               # padded idx count per call (4224)
    QCOL = n_col // 8                # onehot chunk (2048 cols)
    nslots = na // 128

    nc = bacc.Bacc("TRN2", target_bir_lowering=False, debug=True,
                   num_swdge_queues=4)
    _qctr = [0]

    def _nextq():
        q = _qctr[0] % 4
        _qctr[0] += 1
        return q

    ent = nc.dram_tensor("ent", [N_ENTITY, DIM], f32, kind="ExternalInput")
    ent16 = nc.dram_tensor("ent16", [N_ENTITY, 2 * DIM], f16, kind="ExternalInput")
    w1h = nc.dram_tensor("w1h", [64, 128], f16, kind="ExternalInput")
    r1p = nc.dram_tensor("r1p", [N_RELATION, 128], f16, kind="ExternalInput")
    w2 = nc.dram_tensor("w2", [64, 64], f16, kind="ExternalInput")
    wflip = nc.dram_tensor("wflip", [128, 65], f16, kind="ExternalInput")
    items16 = nc.dram_tensor("items16", [128, max(b_core // 16, 1)], i16,
                             kind="ExternalInput")
    idxA, idxh, idxt, r1h = {}, {}, {}, {}
    for s in range(2):
        idxA[s] = nc.dram_tensor(f"idxA{s}", [128, na // 16], i16, kind="ExternalInput")
    for k in range(4):
        idxh[k] = nc.dram_tensor(f"idxh{k}", [128, NH * GQP // 16], i16,
                                 kind="ExternalInput")
        idxt[k] = nc.dram_tensor(f"idxt{k}", [128, NH * GQP // 16], i16,
                                 kind="ExternalInput")
        r1h[k] = nc.dram_tensor(f"r1h{k}", [N_RELATION, n_col], f16, kind="ExternalInput")
    scores_hbm = nc.dram_tensor("scores", [b_core], f32, kind="ExternalOutput")

    # ---- phase-A call list (shared by both stages) ----
    acalls = []
    off = 0
    for w in range(nwin):
        nw_ = caps[w]
        nsplit = max(1, -(-nw_ // 3968))
        step = -(-(-(-nw_ // nsplit)) // 128) * 128
        done = 0
        while done < nw_:
            nn = min(step, nw_ - done)
            acalls.append((w, off + done, nn))
            done += nn
        off += nw_
    NPIECE = 16
    bounds = [0]
    for pi in range(1, NPIECE):
        tgt = na * pi // NPIECE
        bb = 0
        for (_, o2, nn) in acalls:
            if o2 >= tgt:
                break
            bb = o2 + nn
        bounds.append(bb)
    bounds.append(na)
    piece_slots = max(-(-(bounds[pi + 1] - bounds[pi]) // 128)
                      for pi in range(NPIECE))

    with tile.TileContext(nc) as tc:
        with (
            tc.tile_pool(name="dram", bufs=1, space="DRAM") as dp,
            tc.tile_pool(name="const", bufs=1) as cp,
            tc.tile_pool(name="apool", bufs=2) as ap,
            tc.tile_pool(name="bg", bufs=4) as bg,
            tc.tile_pool(name="gsp", bufs=2) as gsp,
            tc.tile_pool(name="rhp", bufs=2) as rhp,
            tc.tile_pool(name="wp", bufs=2) as wp,
            tc.tile_pool(name="sp1", bufs=1) as sp1,
            tc.tile_pool(name="iap", bufs=2) as iap,
            tc.tile_pool(name="ihp", bufs=2) as ihp,
            tc.tile_pool(name="eup", bufs=2) as eup,
            tc.tile_pool(name="kp", bufs=1) as kp,
            tc.tile_pool(name="psP1", bufs=2, space="PSUM") as psP1,
            tc.tile_pool(name="psP2", bufs=2, space="PSUM") as psP2,
            tc.tile_pool(name="psPF", bufs=2, space="PSUM") as psPF,
        ):
            # partition-major staging: token j = (pos%128)*nslots + pos//128;
            # rows stay 256B-linear in j while phase-A writes become 128 big
            # contiguous descriptors.  One stage per tower (shared by layers).
            stage = [dp.tile([128, nslots, 2 * DIM], f16, name=f"stage{s}")
                     for s in range(2)]
            stage_rows = [s[:].rearrange("p s d -> (p s) d") for s in stage]

            ident = cp.tile([128, 128], f32)
            make_identity(nc, ident[:])
            w1h_sb = cp.tile([64, 128], f16)
            nc.sync.dma_start(out=w1h_sb[:], in_=w1h[:])
            r1p_sb = cp.tile([N_RELATION, 128], f16)
            nc.sync.dma_start(out=r1p_sb[:], in_=r1p[:])
            # w2 lives on partitions 64:128 so the p2 matmul's lhsT base
            # partition matches rhs = relu1[64:128].
            w2_sb = cp.tile([128, 64], f16)
            nc.sync.dma_start(out=w2_sb[64:128, :], in_=w2[:])
            wflip_sb = cp.tile([128, 65], f16)
            nc.sync.dma_start(out=wflip_sb[:], in_=wflip[:])

            items_sb = cp.tile([128, max(b_core // 16, 1)], i16)
            nc.sync.dma_start(out=items_sb[:], in_=items16[:])
            iorig = kp.tile([128, NB, DIM], f32)
            nc.gpsimd.dma_gather(
                out_ap=iorig[:], in_ap=ent[:], idxs_ap=items_sb[:],
                num_idxs=b_core, num_idxs_reg=b_core, elem_size=DIM,
                queue_num=_nextq(), single_packet=False)

            def phase_a(s):
                ia = iap.tile([128, na // 16], i16, name=f"ia{s}", tag="ia")
                nc.sync.dma_start(out=ia[:], in_=idxA[s][:])
                for piece in range(NPIECE):
                    p_lo, p_hi = bounds[piece], bounds[piece + 1]
                    if p_lo >= p_hi:
                        continue
                    a16 = ap.tile([128, piece_slots, 2 * DIM], f16,
                                  name=f"a16_{s}_{piece}", tag="a16")
                    for (w, o2, nn) in acalls:
                        if o2 >= p_hi or o2 + nn <= p_lo:
                            continue
                        assert o2 >= p_lo and o2 + nn <= p_hi, "call straddles piece"
                        ol = o2 - p_lo
                        nc.gpsimd.dma_gather(
                            out_ap=a16[:, ol // 128:(ol + nn) // 128, :],
                            in_ap=ent16[WIN * w: min(WIN * (w + 1), N_ENTITY), :],
                            idxs_ap=ia[:, o2 // 16:(o2 + nn) // 16],
                            num_idxs=nn, num_idxs_reg=nn, elem_size=2 * DIM,
                            queue_num=_nextq(), single_packet=False)
                    ns = (p_hi - p_lo) // 128
                    # engine-mediated relay: SWDGE<->HWDGE ordering is only
                    # reliable through engine ops, so copy the real half into
                    # a16b (scalar) and let the staging DMA read that.
                    a16b = ap.tile([128, piece_slots, 2 * DIM], f16,
                                   name=f"a16b_{s}_{piece}", tag="a16b")
                    nc.scalar.activation(out=a16b[:, 0:ns, 0:DIM],
                                         in_=a16[:, 0:ns, 0:DIM], func=AF.Copy)
                    nc.sync.dma_start(
                        out=stage[s][:, p_lo // 128:p_hi // 128, :],
                        in_=a16b[:, 0:ns, :])

            otl = [None] * 4            # per-TL [128, NB, DIM] outputs
            uob = None                  # user-origin batch-major

            phase_a(0)

            for k, (tw, l) in enumerate(TL_LIST):
                srows = stage_rows[k // 2][BASE_TOK:na]
                ih = ihp.tile([128, NH * GQP // 16], i16, name=f"ih{k}", tag="ih")
                nc.sync.dma_start(out=ih[:], in_=idxh[k][:])
                it = ihp.tile([128, NH * GQP // 16], i16, name=f"it{k}", tag="it")
                nc.sync.dma_start(out=it[:], in_=idxt[k][:])

                E = eup.tile([128, T, NB], f32, name=f"E{k}", tag="E")
                U = eup.tile([128, NB, DIM], f32, name=f"U{k}", tag="U")
                if tw == "u" and l == 0:
                    xsum = kp.tile([64, b_core], f32)

                hxs, tes = [], []
                for hf in range(NH):
                    ibase = hf * GQP
                    hx = bg.tile([128, 1, GQP], f16, name=f"hx{k}_{hf}", tag="hx")
                    te = bg.tile([128, GQP // 128, 2 * DIM], f16,
                                 name=f"te{k}_{hf}", tag="te")
                    nc.gpsimd.dma_gather(
                        out_ap=hx[:],
                        in_ap=srows,
                        idxs_ap=ih[:, ibase // 16:(ibase + GQP) // 16],
                        num_idxs=GQP, num_idxs_reg=GQP, elem_size=2 * DIM,
                        transpose=True,
                        queue_num=_nextq(), single_packet=False)
                    nc.gpsimd.dma_gather(
                        out_ap=te[:],
                        in_ap=srows,
                        idxs_ap=it[:, ibase // 16:(ibase + GQP) // 16],
                        num_idxs=GQP, num_idxs_reg=GQP, elem_size=2 * DIM,
                        queue_num=_nextq(), single_packet=False)
                    hxs.append(hx)
                    tes.append(te)

                if k == 0:
                    phase_a(1)

                for hf in range(NH):
                    cbase = hf * GQ
                    hx, te = hxs[hf], tes[hf]
                    hxf = hx[:].rearrange("p a b -> p (a b)")
                    gs = gsp.tile([128, TH, NB, 65], f16, name=f"gs{k}_{hf}", tag="gs")

                    for tt in range(TH):
                        ccol = tt * b_core              # col offset within half
                        if ccol % QCOL == 0:
                            roh = rhp.tile([N_RELATION, QCOL], f16,
                                           name=f"roh{k}_{hf}_{ccol // QCOL}", tag="roh")
                            nc.sync.dma_start(
                                out=roh[:],
                                in_=r1h[k][:, cbase + ccol: cbase + ccol + QCOL])
                        p1 = psP1.tile([128, b_core], f32, space="PSUM",
                                       name="p1", tag="p1")
                        nc.tensor.matmul(out=p1[:], lhsT=w1h_sb[:],
                                         rhs=hxf[0:64, ccol:ccol + b_core],
                                         start=True, stop=False)
                        nc.tensor.matmul(out=p1[:], lhsT=r1p_sb[:],
                                         rhs=roh[:, ccol % QCOL:ccol % QCOL + b_core],
                                         start=False, stop=True)
                        # one relu over all 128 partitions: rows 0:64 = gate
                        # relu1, rows 64:128 = att relu1 (consumed by p2).
                        relu1 = wp.tile([128, b_core], f16, name="relu1", tag="relu1")
                        nc.vector.tensor_scalar(
                            out=relu1[:], in0=p1[:], scalar1=0.0,
                            scalar2=None, op0=ALU.max)
                        p2 = psP2.tile([64, b_core], f32, space="PSUM",
                                       name="p2", tag="p2")
                        nc.tensor.matmul(out=p2[:], lhsT=w2_sb[64:128, :],
                                         rhs=relu1[64:128, :], start=True, stop=True)
                        nc.scalar.activation(out=relu1[64:128, :], in_=p2[:],
                                             func=AF.Relu)
                        pf = psPF.tile([128, NB, 65], f32, space="PSUM",
                                       name="pf", tag="pf")
                        for q in range(NB):
                            nc.tensor.matmul(out=pf[:, q, :],
                                             lhsT=relu1[:, q * 128:(q + 1) * 128],
                                             rhs=wflip_sb[:], start=True, stop=True)
                        nc.scalar.activation(out=gs[:, tt, :, :], in_=pf[:],
                                             func=AF.Sigmoid)

                    # ---- per-half epilogue ----
                    nc.scalar.activation(out=E[:, hf * TH:(hf + 1) * TH, :],
                                         in_=gs[:, :, :, 64], func=AF.Exp)
                    prod = wp.tile([128, TH, NB, DIM], f16, name="prod", tag="prod")
                    nc.vector.tensor_tensor(
                        out=prod[:],
                        in0=te[:, 0:GQ // 128, :].rearrange(
                            "p (t q) d -> p t q d", q=NB)[:, :, :, 0:DIM],
                        in1=gs[:, :, :, 0:DIM], op=ALU.mult)
                    nc.vector.tensor_tensor(
                        out=prod[:], in0=prod[:],
                        in1=E[:, hf * TH:(hf + 1) * TH, :, None]
                            .to_broadcast([128, TH, NB, DIM]),
                        op=ALU.mult)
                    t8 = wp.tile([128, TH // 2, NB, DIM], f16, name="t8", tag="t8")
                    nc.vector.tensor_add(out=t8[:], in0=prod[:, 0:TH // 2, :, :],
                                         in1=prod[:, TH // 2:TH, :, :])
                    nc.vector.tensor_add(out=t8[:, 0:2, :, :], in0=t8[:, 0:2, :, :],
                                         in1=t8[:, 2:4, :, :])
                    if hf == 0:
                        nc.vector.tensor_add(out=U[:], in0=t8[:, 0, :, :],
                                             in1=t8[:, 1, :, :])
                    else:
                        nc.vector.tensor_add(out=t8[:, 0, :, :],
                                             in0=t8[:, 0, :, :],
                                             in1=t8[:, 1, :, :])
                        nc.vector.tensor_add(out=U[:], in0=U[:],
                                             in1=t8[:, 0, :, :])

                    if tw == "u" and l == 0:
                        # user-origin: sum x over t (feature-major halves)
                        xv = hxf[0:64, 0:GQ].rearrange("p (t b) -> p t b", t=TH)
                        x8 = sp1.tile([64, TH // 2, b_core], f32, name="x8", tag="x8")
                        nc.vector.tensor_add(out=x8[:], in0=xv[:, 0:TH // 2, :],
                                             in1=xv[:, TH // 2:TH, :])
                        nc.vector.tensor_add(out=x8[:, 0:2, :], in0=x8[:, 0:2, :],
                                             in1=x8[:, 2:4, :])
                        if hf == 0:
                            nc.vector.tensor_add(out=xsum[:], in0=x8[:, 0, :],
                                                 in1=x8[:, 1, :])
                        else:
                            nc.vector.tensor_add(out=x8[:, 0, :],
                                                 in0=x8[:, 0, :],
                                                 in1=x8[:, 1, :])
                            nc.vector.tensor_add(out=xsum[:], in0=xsum[:],
                                                 in1=x8[:, 0, :])

                # ---- TL epilogue: softmax denom + normalized output ----
                e8 = wp.tile([128, 8, NB], f32, name="e8", tag="e8")
                nc.vector.tensor_add(out=e8[:], in0=E[:, 0:8, :], in1=E[:, 8:16, :])
                nc.vector.tensor_add(out=e8[:], in0=e8[:], in1=E[:, 16:24, :])
                nc.vector.tensor_add(out=e8[:], in0=e8[:], in1=E[:, 24:32, :])
                e2 = wp.tile([128, 2, NB], f32, name="e2", tag="e2")
                nc.vector.tensor_add(out=e2[:], in0=e8[:, 0:2, :], in1=e8[:, 2:4, :])
                nc.vector.tensor_add(out=e2[:], in0=e2[:], in1=e8[:, 4:6, :])
                nc.vector.tensor_add(out=e2[:], in0=e2[:], in1=e8[:, 6:8, :])
                e1 = wp.tile([128, 1, NB], f32, name="e1", tag="e1")
                nc.vector.tensor_add(out=e1[:], in0=e2[:, 0:1, :], in1=e2[:, 1:2, :])
                rs = wp.tile([128, 1, NB], f32, name="rs", tag="rs")
                nc.vector.reciprocal(out=rs[:], in_=e1[:])
                o = kp.tile([128, NB, DIM], f32, name=f"otl{k}", tag=f"otl{k}")
                nc.vector.tensor_tensor(
                    out=o[:], in0=U[:],
                    in1=rs[:, 0, :, None].to_broadcast([128, NB, DIM]),
                    op=ALU.mult)
                otl[k] = o

                if tw == "u" and l == 0:
                    uob = kp.tile([128, NB, DIM], f32)
                    for q in range(NB):
                        ups = psPF.tile([128, DIM], f32, space="PSUM",
                                        name="ups", tag="ups")
                        nc.tensor.transpose(out=ups[:],
                                            in_=xsum[:, q * 128:(q + 1) * 128],
                                            identity=ident[0:64, 0:64])
                        nc.scalar.activation(out=uob[:, q, :], in_=ups[:],
                                             func=AF.Copy)

            # ---- scores ----
            # otl values are unnormalized-by-2: out = U * (1/S); the gate's
            # factor 2 enters squared (u and i towers) => multiply by 4.
            macc = sp1.tile([128, NB, DIM], f32, name="macc", tag="macc")
            nc.vector.tensor_tensor(out=macc[:], in0=uob[:], in1=iorig[:],
                                    op=ALU.mult)
            nc.vector.tensor_scalar(out=macc[:], in0=macc[:], scalar1=1.0 / T,
                                    scalar2=None, op0=ALU.mult)
            for ku, ki in ((0, 2), (1, 3)):
                mu = sp1.tile([128, NB, DIM], f32, name="mu", tag="mu")
                nc.vector.tensor_tensor(out=mu[:], in0=otl[ku][:], in1=otl[ki][:],
                                        op=ALU.mult)
                nc.vector.tensor_scalar(out=mu[:], in0=mu[:], scalar1=4.0,
                                        scalar2=None, op0=ALU.mult)
                nc.vector.tensor_add(out=macc[:], in0=macc[:], in1=mu[:])
            ssum = wp.tile([128, NB, 1], f32, name="ssum", tag="ssum")
            nc.vector.tensor_reduce(out=ssum[:], in_=macc[:], axis=AX.X,
                                    op=ALU.add)
            sc_all = kp.tile([128, NB], f32)
            nc.scalar.activation(out=sc_all[:], in_=ssum[:, :, 0], func=AF.Sigmoid)
            nc.sync.dma_start(out=scores_hbm.rearrange("(s p) -> p s", p=128),
                              in_=sc_all[:])
    nc.compile()
    return nc


def _make_in_maps(inputs):
    b_core, n_col, nwin, caps, na = _dims()
    NH = 4
    GQ = b_core * (T // NH)
    ent = np.asarray(inputs["ent_emb"], np.float32)
    rel = np.asarray(inputs["rel_emb"], np.float32)
    att_w1 = np.asarray(inputs["att_w1"], np.float32)
    att_w2 = np.asarray(inputs["att_w2"], np.float32)
    att_w3 = np.asarray(inputs["att_w3"], np.float32)
    gate_w1 = np.asarray(inputs["gate_w1"], np.float32)
    gate_w2 = np.asarray(inputs["gate_w2"], np.float32)
    items = np.asarray(inputs["items"]).astype(np.int64)
    idx6 = {n: np.asarray(inputs[n]).astype(np.int64)
            for n in ("user_h", "user_r", "user_t", "item_h", "item_r", "item_t")}

    ent16_pad = np.zeros((N_ENTITY, 2 * DIM), np.float16)
    ent16_pad[:, 0:DIM] = ent.astype(np.float16)
    w1h = np.concatenate([gate_w1[:DIM], att_w1[:DIM]], axis=1).astype(np.float16)
    r1p = (rel @ np.concatenate([gate_w1[DIM:], att_w1[DIM:]], axis=1)).astype(np.float16)
    wflip = np.zeros((128, 65), np.float16)
    wflip[0:64, 0:64] = gate_w2.astype(np.float16)
    wflip[64:128, 64:65] = att_w3.astype(np.float16)

    in_maps = []
    for c in range(N_CORES):
        sl = slice(c * b_core, (c + 1) * b_core)
        im = {
            "ent": ent, "ent16": ent16_pad,
            "w1h": w1h, "r1p": r1p, "w2": att_w2.astype(np.float16),
            "wflip": wflip,
            "items16": _wrap_idx16(items[sl].astype(np.int16)),
        }
        for s, tw in enumerate(("user", "item")):
            h0 = idx6[f"{tw}_h"][0, sl].T.ravel()
            t0 = idx6[f"{tw}_t"][0, sl].T.ravel()
            h1 = idx6[f"{tw}_h"][1, sl].T.ravel()
            t1 = idx6[f"{tw}_t"][1, sl].T.ravel()
            ia, (lh0, lt0, lh1, lt1) = _host_prep_stage([h0, t0, h1, t1])
            im[f"idxA{s}"] = _wrap_idx16(ia)
            for li, (lh, lt) in enumerate(((lh0, lt0), (lh1, lt1))):
                k = 2 * s + li
                im[f"idxh{k}"] = _wrap_idx16(_pad_calls(lh, GQ))
                im[f"idxt{k}"] = _wrap_idx16(_pad_calls(lt, GQ))
                r = idx6[f"{tw}_r"][li, sl].T.ravel()
                im[f"r1h{k}"] = (np.arange(N_RELATION)[:, None] == r[None, :]
                                 ).astype(np.float16)
        in_maps.append(im)
    return in_maps


def kernel(**inputs):
    global _NC_CACHE
    import os
    from concourse.bass_utils import run_bass_kernel_spmd

    if _NC_CACHE is None:
        _NC_CACHE = _build_program()
    nc = _NC_CACHE
    in_maps = _make_in_maps(inputs)
    trace = bool(int(os.environ.get("CKAN_TRACE", "0")))
    res = run_bass_kernel_spmd(nc, in_maps, core_ids=list(range(N_CORES)),
                               trace=trace)
    if trace and res.exec_time_ns is not None:
        print(f"HW exec time: {res.exec_time_ns} ns")
    if trace and res.instructions_and_trace is not None:
        print(f"trace path: {res.instructions_and_trace[1]}")
    out = np.concatenate([res.results[c]["scores"] for c in range(N_CORES)])
    return out.astype(np.float32)
